# revision 5
# baseline (speedup 1.0000x reference)
"""Distributed Trainium2 Bass kernel for a pre-LN single attention block.

Reference computation (per problem):
    xn  = LayerNorm(x) * ln_scale + ln_bias          x: [4096, 1024]
    qkv = xn @ w_qkv                                 w_qkv: [1024, 3072]
    q, k, v = split(qkv); heads = 16, dim_head = 64
    sim = einsum('ihd,jhd->ijh', q, k) / sqrt(1024)
    attn = softmax(sim, axis=j)
    out = einsum('ijh,jhd->ihd', attn, v) @ w_out + b_out

Sharding: head-parallel (tensor-parallel over heads, per the problem's
sharding hint) with two communication-minimal collectives:
  1. Each core LayerNorms its own 512-row x shard, transposes it via PE,
     and AllGathers the transposed xn (1 MB bf16 per core). Every core
     then holds xn^T for the full sequence.
  2. Each core computes Q,K,V for its OWN HEAD PAIR over all 4096 rows
     (same total matmul FLOPs as a sequence-parallel split), runs
     attention for that pair over all (i, j), and AllToAlls the
     normalized attention outputs (1 MB bf16 per core) so core c ends
     with all 16 heads for rows 512c..512c+511. Output projection is
     then local.
This moves ~8 MB per core on the wire vs ~17.5 MB for the K/V-AllGather
scheme, needs no DRAM streaming of K/V during attention (everything is
SBUF-resident), and both collectives are single ops (one latency floor
each).

Key device-side layout decisions (bf16 compute / f32 accumulate):
  zT_all  [128, 8kt, 4096]  gathered xn^T: contraction dim on partitions
  kT/qT   [128, 4096]       my pair's k^T/q^T: rows 0-63 head even,
                            64-127 head odd (dh=64)
  scores  ST[j, i] via lhsT=K_h^T - softmax reduction lands on the free
          axis of the attn*V matmul output instead of partitions
  vaug    [128 j, 32 jt, 192]  per j-tile [v_even | ones | v_odd]: the
          shared ones block gives each head a contiguous [v_h|ones] /
          [ones|v_h] 128-column lhsT whose attn*V matmul emits sumexp
          rows for free
  out     yT [1024, 512]  transposed output shard; host transposes+concats
"""

import os

import numpy as np

import concourse.bass as bass
import concourse.mybir as mybir
import concourse.tile as tile
from concourse import bacc
from concourse.bass_utils import run_bass_kernel_spmd
from concourse.masks import make_identity

F32 = mybir.dt.float32
BF16 = mybir.dt.bfloat16
AF = mybir.ActivationFunctionType
ALU = mybir.AluOpType

NC_CORES = 8
SEQ = 4096
DIM = 1024
HEADS = 16
DH = 64
S = SEQ // NC_CORES          # 512 rows per core shard
RT = S // 128                # 4 row tiles per shard
KT = DIM // 128              # 8 contraction tiles over dim
JT = SEQ // 128              # 32 key tiles
IB = SEQ // 512              # 8 query blocks of 512
PW = 3 * DH                  # 192: per-pair v-augmented width
LN_EPS = 1e-6
SOFTMAX_SCALE = float(DIM) ** -0.5

ST_GROUP = int(os.environ.get("ATTN_ST_GROUP", "3"))  # j-tiles per exp batch
N_GROUPS = 6 // ST_GROUP                              # PSUM ring groups
# exp routing pattern: 1=ACT, 0=DVE custom poly op
EXP_PATTERN = [int(c) for c in os.environ.get("ATTN_EXP_PATTERN", "1")]
FAST_RECIP = os.environ.get("ATTN_FAST_RECIP", "1") == "1"

# ---- optional custom DVE polynomial exp ----
# exp(s/32) ~= q(s/128)^4 with q a cubic minimax fit of e^u on |u|<=0.55.
# Softmax-invariant up to the fit's ~5e-4 rel err.
_EXP_K = 1.0 / 128.0
_EXP_CA = 1.001210599703198 * _EXP_K
_EXP_CB = 0.5103360114064122 * _EXP_K * _EXP_K
_EXP_CC = 0.16302281484779013 * _EXP_K * _EXP_K * _EXP_K
_EXP_OP = None


def _get_exp_op():
    global _EXP_OP
    if _EXP_OP is not None:
        return _EXP_OP
    import re
    from concourse import dve_ops
    from concourse.dve_spec import C0, C1, C2, One, Spec, Src0, sq

    def _ref(in0, in1, c0, c1, c2):
        s = in0.astype(np.float32)
        q = 1.0 + s * (c0 + s * (c1 + s * c2))
        q = q * q
        return (q * q).astype(np.float32)

    _q = One + Src0 * (C0 + Src0 * (C1 + Src0 * C2))
    op = dve_ops.DveOp(
        "EXP_POLY4_ANT", Spec(body=sq(sq(_q)), reference=_ref),
        subdim=False, uops_sha={},
    )
    if op.name not in dve_ops._SUB_OPCODE_FOR_NAME:
        dve_ops._SUB_OPCODE_FOR_NAME[op.name] = (
            max(dve_ops._SUB_OPCODE_FOR_NAME.values()) + 1
        )
        for ver in ("v3", "v4"):
            try:
                op.compile(ver)
            except Exception as e:
                m = re.search(r'"([0-9a-f]{16})"', str(e))
                if not m:
                    raise
                op.uops_sha[ver] = m.group(1)
                dve_ops._COMPILE_CACHE.pop((op.name, ver), None)
                op.compile(ver)
        dve_ops.OPS.append(op)
        dve_ops.CUSTOM_DVE_SPECS[op.name] = op.spec
    _EXP_OP = op
    return op


def exp_poly4(nc, out, in_):
    return nc.vector._custom_dve(
        _get_exp_op(), out=out, in0=in_,
        s0=_EXP_CA, s1=_EXP_CB, imm2=_EXP_CC,
    )


def build_graph(sim=False, debug=False, reps=1, phases=None):
    phases = phases or os.environ.get("ATTN_PHASES", "all")
    nc = bacc.Bacc(
        "TRN2",
        target_bir_lowering=False,
        debug=False,
        num_devices=NC_CORES,
    )

    x_ext = nc.declare_dram_parameter("x", [S, DIM], F32, isOutput=False)
    wq_ext = nc.declare_dram_parameter("wq", [DIM, 3 * 128], BF16, isOutput=False)
    cv_ext = nc.declare_dram_parameter("cvec", [3 * 128], F32, isOutput=False)
    wo_ext = nc.declare_dram_parameter("wo", [DIM, DIM], BF16, isOutput=False)
    out_ext = nc.declare_dram_parameter("out", [DIM, S], F32, isOutput=True)
    if debug:
        dbg = {
            "dbg_qT": nc.declare_dram_parameter("dbg_qT", [128, SEQ], F32, isOutput=True),
            "dbg_kT": nc.declare_dram_parameter("dbg_kT", [128, SEQ], F32, isOutput=True),
            "dbg_vA": nc.declare_dram_parameter("dbg_vA", [128, JT, PW], F32, isOutput=True),
            "dbg_avn": nc.declare_dram_parameter("dbg_avn", [128, IB, 512], F32, isOutput=True),
            "dbg_avn2": nc.declare_dram_parameter("dbg_avn2", [128, IB, 512], F32, isOutput=True),
        }

    x_ap = x_ext.ap()
    wq_ap = wq_ext.ap()
    cv_ap = cv_ext.ap()
    wo_ap = wo_ext.ap()
    out_ap = out_ext.ap()

    groups = [list(range(NC_CORES))]

    with tile.TileContext(nc) as tc:
      for _rep in range(reps):
        with (
            tc.tile_pool(name="singles", bufs=1) as singles,
            tc.tile_pool(name="dram", bufs=1, space="DRAM") as dram,
        ):
            # ---- constants ----
            ident = singles.tile([128, 128], BF16)
            make_identity(nc, ident)
            eps_sb = singles.tile([128, 1], F32)
            nc.vector.memset(eps_sb, LN_EPS)
            if reps > 1:
                # serialize reps: eps (hence everything) depends on the
                # previous rep's final output write - makes the reps-delta
                # measure single-shot latency instead of pipelined throughput
                dep_t = singles.tile([128, 512], F32, name="dep")
                nc.sync.dma_start(dep_t[0:1, :], out_ap[0:1, :])
                nc.vector.tensor_scalar_mul(dep_t[0:1, 0:1], dep_t[0:1, 0:1], 0.0)
                nc.vector.tensor_tensor(
                    eps_sb[0:1, :], eps_sb[0:1, :], dep_t[0:1, 0:1], ALU.add
                )
            ones_sb = singles.tile([128, 64], BF16)
            nc.gpsimd.memset(ones_sb, 1.0)
            cv_sb = singles.tile([128, 3], F32)
            nc.sync.dma_start(cv_sb, cv_ap.rearrange("(m p) -> p m", p=128))
            wq_sb = singles.tile([128, KT, 3 * 128], BF16)
            for kt in range(KT):
                nc.sync.dma_start(
                    wq_sb[:, kt], wq_ap.rearrange("(k p) m -> p k m", p=128)[:, kt]
                )
            wo_sb = singles.tile([128, KT, DIM], BF16)
            nc.sync.dma_start(wo_sb, wo_ap.rearrange("(k p) n -> p k n", p=128))

            # ---- collective buffers ----
            # NOTE: Shared-addr-space collective outputs hang on this axon
            # terminal; Local works.
            zT_bounce = dram.tile([DIM, S], BF16)
            zT_all = dram.tile([NC_CORES, DIM, S], BF16)
            a2a_in = dram.tile([NC_CORES, 128 * 512], BF16)
            a2a_out = dram.tile([NC_CORES, 128 * 512], BF16)

            # ---- persistent SBUF activations ----
            qT_sb = singles.tile([128, IB, 512], BF16)   # my pair's q^T
            avn_sb = singles.tile([128, IB, 512], BF16)  # normalized attn out ^T

            # ============ Stage A: LayerNorm + transpose (own shard) ======
            stage_a = tc.tile_pool(name="stage_a", bufs=1)
            ab = stage_a.__enter__()
            zT_own = ab.tile([128, KT, S], BF16)         # LN(x)^T (own rows)
            with (
                tc.tile_pool(name="ln", bufs=2) as ln_pool,
                tc.tile_pool(name="ln_ps", bufs=4, space="PSUM") as ln_ps,
            ):
                for rt in range(RT):
                    x_t = ln_pool.tile([128, DIM], F32, tag="x")
                    nc.sync.dma_start(
                        x_t, x_ap.rearrange("(t p) d -> t p d", p=128)[rt]
                    )
                    stats = ln_pool.tile([128, 2, 6], F32, tag="stats")
                    for sg in range(2):
                        nc.vector.bn_stats(stats[:, sg], x_t[:, 512 * sg:512 * (sg + 1)])
                    mv = ln_pool.tile([128, 2], F32, tag="mv")
                    nc.vector.bn_aggr(mv, stats)
                    std = ln_pool.tile([128, 1], F32, tag="std")
                    nc.scalar.activation(std, mv[:, 1:2], AF.Sqrt, bias=eps_sb)
                    rstd = ln_pool.tile([128, 1], F32, tag="rstd")
                    nc.vector.reciprocal(rstd, std)
                    z_t = ln_pool.tile([128, DIM], BF16, tag="z")
                    nc.vector.tensor_scalar(
                        z_t, x_t, mv[:, 0:1], rstd,
                        op0=ALU.subtract, op1=ALU.mult,
                    )
                    for kt in range(KT):
                        ps_t = ln_ps.tile([128, 128], BF16, tag="tps", name="tps")
                        nc.tensor.transpose(ps_t, z_t[:, 128 * kt:128 * (kt + 1)], ident)
                        nc.vector.tensor_copy(
                            out=zT_own[:, kt, 128 * rt:128 * (rt + 1)], in_=ps_t
                        )

            # ============ Stage B: AllGather xn^T ============
            nc.sync.dma_start(
                zT_bounce.rearrange("(k p) i -> p k i", p=128), zT_own
            )
            if sim:
                for c in range(NC_CORES):
                    nc.sync.dma_start(
                        zT_all[c].rearrange("(k p) i -> p k i", p=128), zT_own
                    )
            else:
                nc.gpsimd.collective_compute(
                    "AllGather", ALU.bypass, replica_groups=groups,
                    ins=[zT_bounce[:].opt()], outs=[zT_all[:].opt()],
                )
            stage_a.__exit__(None, None, None)

            if phases == "head":
                # consume the collective output so its completion is timed
                with tc.tile_pool(name="hddbg", bufs=1) as hd:
                    t1 = hd.tile([128, S], BF16, name="hk")
                    nc.sync.dma_start(t1, zT_all[NC_CORES - 1, 0:128, :])
                    t2 = hd.tile([128, S], F32, name="hk2")
                    nc.vector.tensor_copy(out=t2, in_=t1)
                    nc.sync.dma_start(out_ap[0:128, :], t2)

            if phases != "head":
              # ============ Stage C: load zT, qkv for my pair ============
              stage_c = tc.tile_pool(name="stage_c", bufs=1)
              cb = stage_c.__enter__()
              zT_full = cb.tile([128, KT, SEQ], BF16)
              for c in range(NC_CORES):
                  nc.sync.dma_start(
                      zT_full.rearrange("p k (c i) -> p k c i", c=NC_CORES)[:, :, c],
                      zT_all[c].rearrange("(k p) i -> p k i", p=128),
                  )
              kT_sb = cb.tile([128, JT, 128], BF16)      # my pair's k^T
              vaug_sb = cb.tile([128, JT, PW], BF16)     # [v_e | ones | v_o]
              nc.gpsimd.memset(vaug_sb[:, :, DH:2 * DH], 1.0)

              with (
                  tc.tile_pool(name="qkv_ps", bufs=4, space="PSUM") as qkv_ps,
                  tc.tile_pool(name="vtr", bufs=3) as vtr_pool,
                  tc.tile_pool(name="vtr_ps", bufs=4, space="PSUM") as vtr_ps,
              ):
                  def qkv_chunk(m, ib):
                      ps = qkv_ps.tile([128, 512], F32, tag="qkvps", name="qkvps")
                      for kt in range(KT):
                          nc.tensor.matmul(
                              ps,
                              lhsT=wq_sb[:, kt, 128 * m:128 * (m + 1)],
                              rhs=zT_full[:, kt, 512 * ib:512 * (ib + 1)],
                              start=(kt == 0),
                              stop=(kt == KT - 1),
                          )
                      return ps

                  # K chunks (wq cols 128-255)
                  for ib in range(IB):
                      ps = qkv_chunk(1, ib)
                      nc.vector.tensor_scalar_add(
                          out=kT_sb[:, 4 * ib:4 * ib + 4, :],
                          in0=ps.rearrange("p (j e) -> p j e", e=128),
                          scalar1=cv_sb[:, 1:2],
                      )
                  # Q chunks (wq cols 0-127)
                  for ib in range(IB):
                      ps = qkv_chunk(0, ib)
                      nc.vector.tensor_scalar_add(
                          out=qT_sb[:, ib, :], in0=ps, scalar1=cv_sb[:, 0:1]
                      )
                  # V chunks (wq cols 256-383) -> transposed, augmented
                  for ib in range(IB):
                      ps = qkv_chunk(2, ib)
                      vT_t = vtr_pool.tile([128, 512], BF16, tag="vT", name="vT")
                      nc.vector.tensor_scalar_add(
                          out=vT_t, in0=ps, scalar1=cv_sb[:, 2:3]
                      )
                      for r in range(4):
                          jt = 4 * ib + r
                          ps_t = vtr_ps.tile([128, 128], BF16, tag="vtps",
                                             name="vtps")
                          nc.tensor.transpose(
                              ps_t, vT_t[:, 128 * r:128 * (r + 1)], ident
                          )
                          # even head dims -> cols 0-63, odd -> cols 128-191
                          nc.vector.tensor_copy(
                              out=vaug_sb.rearrange("p j (b e) -> p j b e", e=DH)[
                                  :, jt, 0::2, :
                              ],
                              in_=ps_t.rearrange("p (b e) -> p b e", e=DH),
                          )

              if debug:
                  with tc.tile_pool(name="dbgc", bufs=1) as dbgc:
                      def dump_c(name, sb_tile):
                          n = int(np.prod(sb_tile.shape[1:]))
                          t = dbgc.tile([128, JT * PW], F32, tag="d",
                                        name=f"d_{name}")
                          t = t[:, :n]
                          if len(sb_tile.shape) == 3:
                              t = t.rearrange("p (a b) -> p a b",
                                              a=sb_tile.shape[1])
                          nc.vector.tensor_copy(out=t, in_=sb_tile)
                          nc.sync.dma_start(dbg[name].ap(), t)
                      dump_c("dbg_qT", qT_sb.rearrange("p a b -> p (a b)"))
                      dump_c("dbg_kT", kT_sb.rearrange("p a b -> p (a b)"))
                      dump_c("dbg_vA", vaug_sb)

              # ============ Stage D: attention, per query block ============
              with (
                  tc.tile_pool(name="st_ps", bufs=1, space="PSUM") as st_ps_pool,
                  tc.tile_pool(name="av_ps", bufs=1, space="PSUM") as av_ps_pool,
                  tc.tile_pool(name="expt", bufs=int(os.environ.get("ATTN_EXPT_BUFS", "8"))) as expt_pool,
                  tc.tile_pool(name="norm", bufs=2) as norm_pool,
              ):
                  import itertools
                  exp_rr = itertools.cycle(EXP_PATTERN)
                  for b in range(IB):
                      av_A = av_ps_pool.tile([128, 512], F32, tag="avA", name="avA")
                      av_B = av_ps_pool.tile([128, 512], F32, tag="avB", name="avB")
                      av_AB = (av_A, av_B)

                      # interleaved score tile stream, PSUM ring
                      st_group = [None] * N_GROUPS
                      pending = []  # (hh, jt, group, col) awaiting exp+av

                      def flush_group(g, b=b, av_AB=av_AB,
                                      pending=pending, st_group=st_group):
                          cols = [t for t in pending if t[2] == g]
                          ncols = len(cols)
                          expt = expt_pool.tile(
                              [128, ST_GROUP * 512], BF16, tag="expt", name="expt"
                          )
                          if ncols == ST_GROUP and next(exp_rr) == 0:
                              exp_poly4(
                                  nc, expt[:, :512 * ncols],
                                  st_group[g][:, :512 * ncols],
                              )
                          else:
                              nc.scalar.activation(
                                  expt[:, :512 * ncols],
                                  st_group[g][:, :512 * ncols], AF.Exp,
                                  scale=SOFTMAX_SCALE,
                              )
                          for (hh, jt, gg, col) in cols:
                              # even head: [v_h | ones]; odd: [ones | v_h]
                              off = 64 * hh
                              nc.tensor.matmul(
                                  av_AB[hh],
                                  lhsT=vaug_sb[:, jt, off:off + 128],
                                  rhs=expt[:, 512 * col:512 * (col + 1)],
                                  start=(jt == 0),
                                  stop=(jt == JT - 1),
                              )
                          pending[:] = [t for t in pending if t[2] != g]

                      slot = 0
                      for jt in range(JT):
                          for hh in range(2):
                              g, col = slot // ST_GROUP, slot % ST_GROUP
                              if col == 0:
                                  st_group[g] = st_ps_pool.tile(
                                      [128, ST_GROUP * 512], F32,
                                      tag=f"stg{g}", name=f"stg{g}"
                                  )
                              nc.tensor.matmul(
                                  st_group[g][:, 512 * col:512 * (col + 1)],
                                  lhsT=kT_sb[64 * hh:64 * (hh + 1), jt, :],
                                  rhs=qT_sb[64 * hh:64 * (hh + 1), b, :],
                                  start=True, stop=True,
                              )
                              pending.append((hh, jt, g, col))
                              slot += 1
                              if slot % ST_GROUP == 0:
                                  flush_group(g)
                                  slot = slot % (N_GROUPS * ST_GROUP)
                      if pending:
                          flush_group((slot - 1) // ST_GROUP % N_GROUPS)

                      # normalize: avn = av * (1/sumexp) broadcast via PE
                      for hh in range(2):
                          av = av_AB[hh]
                          row = 64 * (1 - hh)   # sumexp row: 64 even, 0 odd
                          tmp_r = norm_pool.tile([128, 512], F32, tag="tmpr",
                                                 name="tmpr")
                          if row == 0 and FAST_RECIP:
                              # approx recip (~51 ULP) is HW-correct at base
                              # partition 0 but returns wrong data at base 64
                              nc.vector.reciprocal_approx_fast(
                                  tmp_r[row:row + 1, :], av[row:row + 1, :]
                              )
                          else:
                              nc.vector.reciprocal(
                                  tmp_r[row:row + 1, :], av[row:row + 1, :]
                              )
                          tmp_rb = norm_pool.tile([128, 512], BF16, tag="tmprb",
                                                  name="tmprb")
                          nc.vector.tensor_copy(
                              out=tmp_rb[row:row + 1, :],
                              in_=tmp_r[row:row + 1, :],
                          )
                          bc = st_ps_pool.tile([128, 512], F32,
                                               tag=f"stg{hh % N_GROUPS}",
                                               name="bc")
                          nc.tensor.matmul(
                              bc[64 * hh:64 * (hh + 1), :],
                              lhsT=ones_sb[row:row + 1, :],
                              rhs=tmp_rb[row:row + 1, :],
                              start=True, stop=True,
                          )
                          bc_sb = norm_pool.tile([128, 512], BF16, tag="bcsb",
                                                 name="bcsb")
                          nc.vector.tensor_copy(
                              out=bc_sb[64 * hh:64 * (hh + 1), :],
                              in_=bc[64 * hh:64 * (hh + 1), :],
                          )
                          nc.vector.tensor_tensor(
                              avn_sb[64 * hh:64 * (hh + 1), b, :],
                              av[64 * hh:64 * (hh + 1), :],
                              bc_sb[64 * hh:64 * (hh + 1), :],
                              ALU.mult,
                          )
                      # ship this block's slab for the AllToAll
                      nc.sync.dma_start(
                          a2a_in[b].rearrange("(p i) -> p i", p=128),
                          avn_sb[:, b, :],
                      )
              stage_c.__exit__(None, None, None)

              if debug:
                  with tc.tile_pool(name="dbgd", bufs=1) as dbgd:
                      t = dbgd.tile([128, IB * 512], F32, name="d_avn")
                      nc.vector.tensor_copy(
                          out=t.rearrange("p (a b) -> p a b", a=IB), in_=avn_sb
                      )
                      nc.sync.dma_start(dbg["dbg_avn"].ap(), t.rearrange(
                          "p (a b) -> p a b", a=IB))

              # ============ Stage E: AllToAll + output projection ============
              if sim:
                  for j in range(NC_CORES):
                      nc.sync.dma_start(a2a_out[j], a2a_in[j])
              else:
                  nc.gpsimd.collective_compute(
                      "AllToAll", ALU.bypass, replica_groups=groups,
                      ins=[a2a_in[:].opt()], outs=[a2a_out[:].opt()],
                  )

              if phases == "all":
                with (
                    tc.tile_pool(name="out_ps", bufs=4, space="PSUM") as out_ps_pool,
                    tc.tile_pool(name="out_sb", bufs=3) as out_sb_pool,
                    tc.tile_pool(name="avn2", bufs=1) as avn2_pool,
                ):
                  avn2_sb = avn2_pool.tile([128, KT, S], BF16)
                  for j in range(NC_CORES):
                      nc.sync.dma_start(
                          avn2_sb[:, j, :],
                          a2a_out[j].rearrange("(p i) -> p i", p=128),
                      )
                  if debug:
                      t = avn2_pool.tile([128, IB, 512], F32, name="d_avn2")
                      nc.vector.tensor_copy(out=t, in_=avn2_sb)
                      nc.sync.dma_start(dbg["dbg_avn2"].ap(), t)
                  for n in range(KT):
                      ps = out_ps_pool.tile([128, S], F32, tag="outps", name="outps")
                      for kt in range(KT):
                          nc.tensor.matmul(
                              ps,
                              lhsT=wo_sb[:, kt, 128 * n:128 * (n + 1)],
                              rhs=avn2_sb[:, kt, :],
                              start=(kt == 0),
                              stop=(kt == KT - 1),
                          )
                      y_t = out_sb_pool.tile([128, S], F32, tag="yt", name="yt")
                      nc.vector.tensor_copy(out=y_t, in_=ps)
                      nc.sync.dma_start(
                          out_ap.rearrange("(n p) i -> n p i", p=128)[n], y_t
                      )

    nc.compile()
    return nc


_GRAPH_CACHE = {}


def _get_graph():
    if "nc" not in _GRAPH_CACHE:
        _GRAPH_CACHE["nc"] = build_graph()
    return _GRAPH_CACHE["nc"]


def make_in_maps(x, ln_scale, ln_bias, w_qkv, w_out):
    import ml_dtypes

    W = (np.asarray(ln_scale, np.float32)[:, None]
         * np.asarray(w_qkv, np.float32))
    cvec = (np.asarray(ln_bias, np.float32) @ np.asarray(w_qkv, np.float32)).astype(
        np.float32
    )
    wo = np.asarray(w_out, np.float32).astype(ml_dtypes.bfloat16)
    x = np.asarray(x, np.float32)
    in_maps = []
    for c in range(NC_CORES):
        cols = np.r_[128 * c:128 * c + 128,
                     DIM + 128 * c:DIM + 128 * c + 128,
                     2 * DIM + 128 * c:2 * DIM + 128 * c + 128]
        in_maps.append({
            "x": np.ascontiguousarray(x[c * S:(c + 1) * S]),
            "wq": np.ascontiguousarray(W[:, cols]).astype(ml_dtypes.bfloat16),
            "cvec": np.ascontiguousarray(cvec[cols]),
            "wo": wo,
        })
    return in_maps


def kernel(x, ln_scale, ln_bias, w_qkv, w_out, b_out, **run_kwargs):
    in_maps = make_in_maps(x, ln_scale, ln_bias, w_qkv, w_out)
    nc = _get_graph()
    res = run_bass_kernel_spmd(
        nc, in_maps, core_ids=list(range(NC_CORES)), **run_kwargs
    )
    _GRAPH_CACHE["last_results"] = res
    out = np.concatenate(
        [np.asarray(r["out"]).T for r in res.results], axis=0
    )
    return (out + np.asarray(b_out, np.float32)).astype(np.float32)


# revision 15
# speedup vs baseline: 1.1579x; 1.1579x over previous
"""Distributed Trainium2 Bass kernel for a pre-LN single attention block.

Reference computation (per problem):
    xn  = LayerNorm(x) * ln_scale + ln_bias          x: [4096, 1024]
    qkv = xn @ w_qkv                                 w_qkv: [1024, 3072]
    q, k, v = split(qkv); heads = 16, dim_head = 64
    sim = einsum('ihd,jhd->ijh', q, k) / sqrt(1024)
    attn = softmax(sim, axis=j)
    out = einsum('ijh,jhd->ihd', attn, v) @ w_out + b_out

Sharding: head-parallel attention (tensor-parallel over heads, per the
problem's sharding hint) fed by sequence-parallel projections, with two
communication-minimal AllToAlls:
  1. Each core LayerNorms its own 512-row x shard, computes Q,K,V for
     ALL 16 heads of those rows (the same total matmul FLOPs as any
     other split), and AllToAlls the per-head-pair {q,k,v} slabs
     (3 MB bf16 buffer -> ~2.6 MB on the wire). Core h then holds
     q,k,v for head pair h over the full 4096-row sequence.
  2. Core h runs attention for its pair over all (i, j) - everything
     SBUF-resident, no K/V DMA streaming - and AllToAlls the
     normalized attention outputs (1 MB bf16) so core c ends with all
     16 heads for rows 512c..512c+511. Output projection is local.
Total wire traffic is ~3.5 MB per core vs ~17.5 MB for the K/V
AllGather scheme and ~8 MB for an xn-AllGather scheme, and the big
collective sits right after a dense 40 us matmul block it can drain
behind.

Key device-side layout decisions (bf16 compute / f32 accumulate):
  zT_all  [128, 8kt, 4096]  gathered xn^T: contraction dim on partitions
  kT/qT   [128, 4096]       my pair's k^T/q^T: rows 0-63 head even,
                            64-127 head odd (dh=64)
  scores  ST[j, i] via lhsT=K_h^T - softmax reduction lands on the free
          axis of the attn*V matmul output instead of partitions
  vaug    [128 j, 32 jt, 192]  per j-tile [v_even | ones | v_odd]: the
          shared ones block gives each head a contiguous [v_h|ones] /
          [ones|v_h] 128-column lhsT whose attn*V matmul emits sumexp
          rows for free
  out     yT [1024, 512]  transposed output shard; host transposes+concats
"""

import os

import numpy as np

import concourse.bass as bass
import concourse.mybir as mybir
import concourse.tile as tile
from concourse import bacc
from concourse.bass_utils import run_bass_kernel_spmd
from concourse.masks import make_identity

F32 = mybir.dt.float32
BF16 = mybir.dt.bfloat16
AF = mybir.ActivationFunctionType
ALU = mybir.AluOpType

NC_CORES = 8
SEQ = 4096
DIM = 1024
HEADS = 16
DH = 64
S = SEQ // NC_CORES          # 512 rows per core shard
RT = S // 128                # 4 row tiles per shard
KT = DIM // 128              # 8 contraction tiles over dim
JT = SEQ // 128              # 32 key tiles
IB = SEQ // 512              # 8 query blocks of 512
PW = 3 * DH                  # 192: per-pair v-augmented width
LN_EPS = 1e-6
SOFTMAX_SCALE = float(DIM) ** -0.5

ST_GROUP = int(os.environ.get("ATTN_ST_GROUP", "3"))  # j-tiles per exp batch
N_GROUPS = 6 // ST_GROUP                              # PSUM ring groups
# exp routing pattern: 1=ACT, 0=DVE custom poly op
EXP_PATTERN = [int(c) for c in os.environ.get("ATTN_EXP_PATTERN", "1")]
FAST_RECIP = os.environ.get("ATTN_FAST_RECIP", "1") == "1"

# ---- optional custom DVE polynomial exp ----
# exp(s/32) ~= q(s/128)^4 with q a cubic minimax fit of e^u on |u|<=0.55.
# Softmax-invariant up to the fit's ~5e-4 rel err.
_EXP_K = 1.0 / 128.0
_EXP_CA = 1.001210599703198 * _EXP_K
_EXP_CB = 0.5103360114064122 * _EXP_K * _EXP_K
_EXP_CC = 0.16302281484779013 * _EXP_K * _EXP_K * _EXP_K
_EXP_OP = None


def _get_exp_op():
    global _EXP_OP
    if _EXP_OP is not None:
        return _EXP_OP
    import re
    from concourse import dve_ops
    from concourse.dve_spec import C0, C1, C2, One, Spec, Src0, sq

    def _ref(in0, in1, c0, c1, c2):
        s = in0.astype(np.float32)
        q = 1.0 + s * (c0 + s * (c1 + s * c2))
        q = q * q
        return (q * q).astype(np.float32)

    _q = One + Src0 * (C0 + Src0 * (C1 + Src0 * C2))
    op = dve_ops.DveOp(
        "EXP_POLY4_ANT", Spec(body=sq(sq(_q)), reference=_ref),
        subdim=False, uops_sha={},
    )
    if op.name not in dve_ops._SUB_OPCODE_FOR_NAME:
        dve_ops._SUB_OPCODE_FOR_NAME[op.name] = (
            max(dve_ops._SUB_OPCODE_FOR_NAME.values()) + 1
        )
        for ver in ("v3", "v4"):
            try:
                op.compile(ver)
            except Exception as e:
                m = re.search(r'"([0-9a-f]{16})"', str(e))
                if not m:
                    raise
                op.uops_sha[ver] = m.group(1)
                dve_ops._COMPILE_CACHE.pop((op.name, ver), None)
                op.compile(ver)
        dve_ops.OPS.append(op)
        dve_ops.CUSTOM_DVE_SPECS[op.name] = op.spec
    _EXP_OP = op
    return op


def exp_poly4(nc, out, in_):
    return nc.vector._custom_dve(
        _get_exp_op(), out=out, in0=in_,
        s0=_EXP_CA, s1=_EXP_CB, imm2=_EXP_CC,
    )


def build_graph(sim=False, debug=False, reps=1, phases=None):
    phases = phases or os.environ.get("ATTN_PHASES", "all")
    nc = bacc.Bacc(
        "TRN2",
        target_bir_lowering=False,
        debug=False,
        num_devices=NC_CORES,
    )

    x_ext = nc.declare_dram_parameter("x", [S, DIM], F32, isOutput=False)
    wq_ext = nc.declare_dram_parameter("wq", [DIM, 3 * DIM], BF16, isOutput=False)
    cv_ext = nc.declare_dram_parameter("cvec", [3 * DIM], F32, isOutput=False)
    wo_ext = nc.declare_dram_parameter("wo", [DIM, DIM], BF16, isOutput=False)
    out_ext = nc.declare_dram_parameter("out", [DIM, S], F32, isOutput=True)
    if debug:
        dbg = {
            "dbg_qT": nc.declare_dram_parameter("dbg_qT", [128, SEQ], F32, isOutput=True),
            "dbg_kT": nc.declare_dram_parameter("dbg_kT", [128, SEQ], F32, isOutput=True),
            "dbg_vA": nc.declare_dram_parameter("dbg_vA", [128, JT, PW], F32, isOutput=True),
            "dbg_avn": nc.declare_dram_parameter("dbg_avn", [128, IB, 512], F32, isOutput=True),
            "dbg_avn2": nc.declare_dram_parameter("dbg_avn2", [128, IB, 512], F32, isOutput=True),
        }

    x_ap = x_ext.ap()
    wq_ap = wq_ext.ap()
    cv_ap = cv_ext.ap()
    wo_ap = wo_ext.ap()
    out_ap = out_ext.ap()

    groups = [list(range(NC_CORES))]

    with tile.TileContext(nc) as tc:
      for _rep in range(reps):
        with (
            tc.tile_pool(name="singles", bufs=1) as singles,
            tc.tile_pool(name="dram", bufs=1, space="DRAM") as dram,
        ):
            # ---- constants ----
            ident = singles.tile([128, 128], BF16)
            make_identity(nc, ident)
            eps_sb = singles.tile([128, 1], F32)
            nc.vector.memset(eps_sb, LN_EPS)
            if reps > 1:
                # serialize reps: eps (hence everything) depends on the
                # previous rep's final output write - makes the reps-delta
                # measure single-shot latency instead of pipelined throughput
                dep_t = singles.tile([128, 512], F32, name="dep")
                nc.sync.dma_start(dep_t[0:1, :], out_ap[0:1, :])
                nc.vector.tensor_scalar_mul(dep_t[0:1, 0:1], dep_t[0:1, 0:1], 0.0)
                nc.vector.tensor_tensor(
                    eps_sb[0:1, :], eps_sb[0:1, :], dep_t[0:1, 0:1], ALU.add
                )
            ones_sb = singles.tile([128, 64], BF16)
            nc.gpsimd.memset(ones_sb, 1.0)
            cv_sb = singles.tile([128, 24], F32)
            nc.sync.dma_start(cv_sb, cv_ap.rearrange("(m p) -> p m", p=128))
            wo_sb = singles.tile([128, KT, DIM], BF16)
            nc.sync.dma_start(wo_sb, wo_ap.rearrange("(k p) n -> p k n", p=128))

            # ---- collective buffers ----
            # NOTE: Shared-addr-space collective outputs hang on this axon
            # terminal; Local works.
            # qkv A2A slab layout per destination pair h:
            #   part 0: q [128 dims, 512 own rows]   (p-major, p*512+i)
            #   part 1: k [128 dims, 512 own rows]   (p-major)
            #   part 2: v [4 rt, 128 j, 128 dims]    (natural, r*16384+p*128+d)
            qkv_a2a_in = dram.tile([NC_CORES, 3, 128 * 512], BF16)
            qkv_a2a_out = dram.tile([NC_CORES, 3, 128 * 512], BF16)
            a2a_in = dram.tile([NC_CORES, 128 * 512], BF16)
            a2a_out = dram.tile([NC_CORES, 128 * 512], BF16)

            # ---- persistent SBUF activations ----
            qT_sb = singles.tile([128, IB, 512], BF16)   # my pair's q^T
            avn_sb = singles.tile([128, IB, 512], BF16)  # normalized attn out ^T

            # ============ Stage A: LayerNorm + transpose (own shard) ======
            stage_a = tc.tile_pool(name="stage_a", bufs=1)
            ab = stage_a.__enter__()
            zT_own = ab.tile([128, KT, S], BF16)         # LN(x)^T (own rows)
            wq_sb = ab.tile([128, KT, 3 * DIM], BF16)
            for kt in range(KT):
                nc.sync.dma_start(
                    wq_sb[:, kt], wq_ap.rearrange("(k p) m -> p k m", p=128)[:, kt]
                )
            with (
                tc.tile_pool(name="ln", bufs=2) as ln_pool,
                tc.tile_pool(name="ln_ps", bufs=4, space="PSUM") as ln_ps,
            ):
                for rt in range(RT):
                    x_t = ln_pool.tile([128, DIM], F32, tag="x")
                    nc.sync.dma_start(
                        x_t, x_ap.rearrange("(t p) d -> t p d", p=128)[rt]
                    )
                    stats = ln_pool.tile([128, 2, 6], F32, tag="stats")
                    for sg in range(2):
                        nc.vector.bn_stats(stats[:, sg], x_t[:, 512 * sg:512 * (sg + 1)])
                    mv = ln_pool.tile([128, 2], F32, tag="mv")
                    nc.vector.bn_aggr(mv, stats)
                    std = ln_pool.tile([128, 1], F32, tag="std")
                    nc.scalar.activation(std, mv[:, 1:2], AF.Sqrt, bias=eps_sb)
                    rstd = ln_pool.tile([128, 1], F32, tag="rstd")
                    nc.vector.reciprocal(rstd, std)
                    z_t = ln_pool.tile([128, DIM], BF16, tag="z")
                    nc.vector.tensor_scalar(
                        z_t, x_t, mv[:, 0:1], rstd,
                        op0=ALU.subtract, op1=ALU.mult,
                    )
                    for kt in range(KT):
                        ps_t = ln_ps.tile([128, 128], BF16, tag="tps", name="tps")
                        nc.tensor.transpose(ps_t, z_t[:, 128 * kt:128 * (kt + 1)], ident)
                        nc.vector.tensor_copy(
                            out=zT_own[:, kt, 128 * rt:128 * (rt + 1)], in_=ps_t
                        )

            # ===== Stage B: qkv for ALL heads on own rows, AllToAll =====
            with (
                tc.tile_pool(name="qkv_ps", bufs=4, space="PSUM") as qkv_ps,
                tc.tile_pool(name="qkv_sb", bufs=4) as qkv_sb,
                tc.tile_pool(name="vtr_ps", bufs=4, space="PSUM") as vtr_ps,
            ):
                def qkv_mtile(m):
                    ps = qkv_ps.tile([128, S], F32, tag="qkvps", name="qkvps")
                    for kt in range(KT):
                        nc.tensor.matmul(
                            ps,
                            lhsT=wq_sb[:, kt, 128 * m:128 * (m + 1)],
                            rhs=zT_own[:, kt, :],
                            start=(kt == 0),
                            stop=(kt == KT - 1),
                        )
                    return ps

                # Q then K m-tiles: straight to the A2A slab
                for part, m0 in ((0, 0), (1, KT)):
                    for h in range(KT):
                        ps = qkv_mtile(m0 + h)
                        o_t = qkv_sb.tile([128, S], BF16, tag="qko", name="qko")
                        nc.vector.tensor_scalar_add(
                            out=o_t, in0=ps, scalar1=cv_sb[:, m0 + h:m0 + h + 1]
                        )
                        nc.sync.dma_start(
                            qkv_a2a_in[h, part].rearrange("(p i) -> p i", p=128),
                            o_t,
                        )
                # V m-tiles: transpose to natural layout, then to slab
                for h in range(KT):
                    m = 2 * KT + h
                    ps = qkv_mtile(m)
                    vT_t = qkv_sb.tile([128, S], BF16, tag="qko", name="vT")
                    nc.vector.tensor_scalar_add(
                        out=vT_t, in0=ps, scalar1=cv_sb[:, m:m + 1]
                    )
                    vn_t = qkv_sb.tile([128, RT, 128], BF16, tag="vn", name="vn")
                    for r in range(RT):
                        ps_t = vtr_ps.tile([128, 128], BF16, tag="vtps",
                                           name="vtps")
                        nc.tensor.transpose(
                            ps_t, vT_t[:, 128 * r:128 * (r + 1)], ident
                        )
                        nc.vector.tensor_copy(out=vn_t[:, r, :], in_=ps_t)
                    nc.sync.dma_start(
                        qkv_a2a_in[h, 2].rearrange("(r p d) -> p r d", p=128, d=128),
                        vn_t,
                    )
            stage_a.__exit__(None, None, None)

            if sim:
                for j in range(NC_CORES):
                    nc.sync.dma_start(qkv_a2a_out[j], qkv_a2a_in[j])
            else:
                nc.gpsimd.collective_compute(
                    "AllToAll", ALU.bypass, replica_groups=groups,
                    ins=[qkv_a2a_in[:].opt()], outs=[qkv_a2a_out[:].opt()],
                )

            if phases == "head":
                # consume the collective output so its completion is timed
                with tc.tile_pool(name="hddbg", bufs=1) as hd:
                    t1 = hd.tile([128, S], BF16, name="hk")
                    nc.sync.dma_start(
                        t1,
                        qkv_a2a_out[NC_CORES - 1, 0].rearrange(
                            "(p i) -> p i", p=128),
                    )
                    t2 = hd.tile([128, S], F32, name="hk2")
                    nc.vector.tensor_copy(out=t2, in_=t1)
                    nc.sync.dma_start(out_ap[0:128, :], t2)

            if phases not in ("head", "all", "noa2a", "noout", "qkv"):
                raise ValueError(phases)
            if phases != "head":
              # ===== Stage C: assemble my pair's qT / kT / vaug =====
              stage_c = tc.tile_pool(name="stage_c", bufs=1)
              cb = stage_c.__enter__()
              kT_sb = cb.tile([128, JT, 128], BF16)      # my pair's k^T
              vaug_sb = cb.tile([128, JT, PW], BF16)     # [v_e | ones | v_o]
              nc.gpsimd.memset(vaug_sb[:, :, DH:2 * DH], 1.0)
              for j in range(NC_CORES):
                  nc.sync.dma_start(
                      qT_sb[:, j, :],
                      qkv_a2a_out[j, 0].rearrange("(p i) -> p i", p=128),
                  )
                  nc.sync.dma_start(
                      kT_sb[:, 4 * j:4 * j + 4, :],
                      qkv_a2a_out[j, 1].rearrange("(p r e) -> p r e", r=RT, e=128),
                  )
                  # v natural: [r, p, d] -> even head dims to cols 0-63,
                  # odd head dims to cols 128-191
                  nc.sync.dma_start(
                      vaug_sb[:, 4 * j:4 * j + 4, 0:DH],
                      qkv_a2a_out[j, 2].rearrange(
                          "(r p d) -> p r d", p=128, d=128)[:, :, 0:DH],
                  )
                  nc.sync.dma_start(
                      vaug_sb[:, 4 * j:4 * j + 4, 2 * DH:3 * DH],
                      qkv_a2a_out[j, 2].rearrange(
                          "(r p d) -> p r d", p=128, d=128)[:, :, DH:2 * DH],
                  )

              if debug:
                  with tc.tile_pool(name="dbgc", bufs=1) as dbgc:
                      def dump_c(name, sb_tile):
                          n = int(np.prod(sb_tile.shape[1:]))
                          t = dbgc.tile([128, JT * PW], F32, tag="d",
                                        name=f"d_{name}")
                          t = t[:, :n]
                          if len(sb_tile.shape) == 3:
                              t = t.rearrange("p (a b) -> p a b",
                                              a=sb_tile.shape[1])
                          nc.vector.tensor_copy(out=t, in_=sb_tile)
                          nc.sync.dma_start(dbg[name].ap(), t)
                      dump_c("dbg_qT", qT_sb.rearrange("p a b -> p (a b)"))
                      dump_c("dbg_kT", kT_sb.rearrange("p a b -> p (a b)"))
                      dump_c("dbg_vA", vaug_sb)

              if phases == "qkv":
                  # consume qkv outputs so their completion is timed
                  with tc.tile_pool(name="qkvdbg", bufs=1) as qd:
                      t2 = qd.tile([128, 512], F32, name="qd2")
                      nc.vector.tensor_copy(
                          out=t2,
                          in_=kT_sb[:, JT - 4:JT, :].rearrange("p j e -> p (j e)"),
                      )
                      nc.vector.tensor_tensor(
                          t2[:, :DH], t2[:, :DH], vaug_sb[:, JT - 3, 0:DH],
                          ALU.add)
                      nc.vector.tensor_tensor(
                          t2, t2, qT_sb[:, IB - 1, :], ALU.add)
                      nc.sync.dma_start(out_ap[0:128, :], t2)
              if phases != "qkv":
               # ============ Stage D: attention, per query block ============
               with (
                  tc.tile_pool(name="st_ps", bufs=1, space="PSUM") as st_ps_pool,
                  tc.tile_pool(name="av_ps", bufs=1, space="PSUM") as av_ps_pool,
                  tc.tile_pool(name="expt", bufs=int(os.environ.get("ATTN_EXPT_BUFS", "8"))) as expt_pool,
                  tc.tile_pool(name="norm", bufs=2) as norm_pool,
              ):
                  import itertools
                  exp_rr = itertools.cycle(EXP_PATTERN)
                  for b in range(IB):
                      av_A = av_ps_pool.tile([128, 512], F32, tag="avA", name="avA")
                      av_B = av_ps_pool.tile([128, 512], F32, tag="avB", name="avB")
                      av_AB = (av_A, av_B)

                      # interleaved score tile stream, PSUM ring
                      st_group = [None] * N_GROUPS
                      pending = []  # (hh, jt, group, col) awaiting exp+av

                      def flush_group(g, b=b, av_AB=av_AB,
                                      pending=pending, st_group=st_group):
                          cols = [t for t in pending if t[2] == g]
                          ncols = len(cols)
                          expt = expt_pool.tile(
                              [128, ST_GROUP * 512], BF16, tag="expt", name="expt"
                          )
                          if ncols == ST_GROUP and next(exp_rr) == 0:
                              exp_poly4(
                                  nc, expt[:, :512 * ncols],
                                  st_group[g][:, :512 * ncols],
                              )
                          else:
                              nc.scalar.activation(
                                  expt[:, :512 * ncols],
                                  st_group[g][:, :512 * ncols], AF.Exp,
                                  scale=SOFTMAX_SCALE,
                              )
                          for (hh, jt, gg, col) in cols:
                              # even head: [v_h | ones]; odd: [ones | v_h]
                              off = 64 * hh
                              nc.tensor.matmul(
                                  av_AB[hh],
                                  lhsT=vaug_sb[:, jt, off:off + 128],
                                  rhs=expt[:, 512 * col:512 * (col + 1)],
                                  start=(jt == 0),
                                  stop=(jt == JT - 1),
                              )
                          pending[:] = [t for t in pending if t[2] != g]

                      slot = 0
                      for jt in range(JT):
                          for hh in range(2):
                              g, col = slot // ST_GROUP, slot % ST_GROUP
                              if col == 0:
                                  st_group[g] = st_ps_pool.tile(
                                      [128, ST_GROUP * 512], F32,
                                      tag=f"stg{g}", name=f"stg{g}"
                                  )
                              nc.tensor.matmul(
                                  st_group[g][:, 512 * col:512 * (col + 1)],
                                  lhsT=kT_sb[64 * hh:64 * (hh + 1), jt, :],
                                  rhs=qT_sb[64 * hh:64 * (hh + 1), b, :],
                                  start=True, stop=True,
                              )
                              pending.append((hh, jt, g, col))
                              slot += 1
                              if slot % ST_GROUP == 0:
                                  flush_group(g)
                                  slot = slot % (N_GROUPS * ST_GROUP)
                      if pending:
                          flush_group((slot - 1) // ST_GROUP % N_GROUPS)

                      # normalize: avn = av * (1/sumexp) broadcast via PE
                      for hh in range(2):
                          av = av_AB[hh]
                          row = 64 * (1 - hh)   # sumexp row: 64 even, 0 odd
                          tmp_r = norm_pool.tile([128, 512], F32, tag="tmpr",
                                                 name="tmpr")
                          if row == 0 and FAST_RECIP:
                              # approx recip (~51 ULP) is HW-correct at base
                              # partition 0 but returns wrong data at base 64
                              nc.vector.reciprocal_approx_fast(
                                  tmp_r[row:row + 1, :], av[row:row + 1, :]
                              )
                          else:
                              nc.vector.reciprocal(
                                  tmp_r[row:row + 1, :], av[row:row + 1, :]
                              )
                          tmp_rb = norm_pool.tile([128, 512], BF16, tag="tmprb",
                                                  name="tmprb")
                          nc.vector.tensor_copy(
                              out=tmp_rb[row:row + 1, :],
                              in_=tmp_r[row:row + 1, :],
                          )
                          bc = st_ps_pool.tile([128, 512], F32,
                                               tag=f"stg{hh % N_GROUPS}",
                                               name="bc")
                          nc.tensor.matmul(
                              bc[64 * hh:64 * (hh + 1), :],
                              lhsT=ones_sb[row:row + 1, :],
                              rhs=tmp_rb[row:row + 1, :],
                              start=True, stop=True,
                          )
                          bc_sb = norm_pool.tile([128, 512], BF16, tag="bcsb",
                                                 name="bcsb")
                          nc.vector.tensor_copy(
                              out=bc_sb[64 * hh:64 * (hh + 1), :],
                              in_=bc[64 * hh:64 * (hh + 1), :],
                          )
                          nc.vector.tensor_tensor(
                              avn_sb[64 * hh:64 * (hh + 1), b, :],
                              av[64 * hh:64 * (hh + 1), :],
                              bc_sb[64 * hh:64 * (hh + 1), :],
                              ALU.mult,
                          )
                      # ship this block's slab for the AllToAll
                      nc.sync.dma_start(
                          a2a_in[b].rearrange("(p i) -> p i", p=128),
                          avn_sb[:, b, :],
                      )
              stage_c.__exit__(None, None, None)

              if debug:
                  with tc.tile_pool(name="dbgd", bufs=1) as dbgd:
                      t = dbgd.tile([128, IB * 512], F32, name="d_avn")
                      nc.vector.tensor_copy(
                          out=t.rearrange("p (a b) -> p a b", a=IB), in_=avn_sb
                      )
                      nc.sync.dma_start(dbg["dbg_avn"].ap(), t.rearrange(
                          "p (a b) -> p a b", a=IB))

              # ============ Stage E: AllToAll + output projection ============
              if phases in ("noout", "all"):
                if sim:
                  for j in range(NC_CORES):
                      nc.sync.dma_start(a2a_out[j], a2a_in[j])
                else:
                  nc.gpsimd.collective_compute(
                      "AllToAll", ALU.bypass, replica_groups=groups,
                      ins=[a2a_in[:].opt()], outs=[a2a_out[:].opt()],
                  )
                if phases == "noout":
                  with tc.tile_pool(name="a2adbg", bufs=1) as ad:
                      t3 = ad.tile([128, 512], BF16, name="ad1")
                      nc.sync.dma_start(
                          t3, a2a_out[NC_CORES - 1].rearrange("(p i) -> p i", p=128))
                      t4 = ad.tile([128, 512], F32, name="ad2")
                      nc.vector.tensor_copy(out=t4, in_=t3)
                      nc.sync.dma_start(out_ap[0:128, :], t4)

              if phases == "all":
                with (
                    tc.tile_pool(name="out_ps", bufs=4, space="PSUM") as out_ps_pool,
                    tc.tile_pool(name="out_sb", bufs=3) as out_sb_pool,
                    tc.tile_pool(name="avn2", bufs=1) as avn2_pool,
                ):
                  avn2_sb = avn2_pool.tile([128, KT, S], BF16)
                  for j in range(NC_CORES):
                      nc.sync.dma_start(
                          avn2_sb[:, j, :],
                          a2a_out[j].rearrange("(p i) -> p i", p=128),
                      )
                  if debug:
                      t = avn2_pool.tile([128, IB, 512], F32, name="d_avn2")
                      nc.vector.tensor_copy(out=t, in_=avn2_sb)
                      nc.sync.dma_start(dbg["dbg_avn2"].ap(), t)
                  for n in range(KT):
                      ps = out_ps_pool.tile([128, S], F32, tag="outps", name="outps")
                      for kt in range(KT):
                          nc.tensor.matmul(
                              ps,
                              lhsT=wo_sb[:, kt, 128 * n:128 * (n + 1)],
                              rhs=avn2_sb[:, kt, :],
                              start=(kt == 0),
                              stop=(kt == KT - 1),
                          )
                      y_t = out_sb_pool.tile([128, S], F32, tag="yt", name="yt")
                      nc.vector.tensor_copy(out=y_t, in_=ps)
                      nc.sync.dma_start(
                          out_ap.rearrange("(n p) i -> n p i", p=128)[n], y_t
                      )

    nc.compile()
    return nc


_GRAPH_CACHE = {}


def _get_graph():
    if "nc" not in _GRAPH_CACHE:
        _GRAPH_CACHE["nc"] = build_graph()
    return _GRAPH_CACHE["nc"]


def make_in_maps(x, ln_scale, ln_bias, w_qkv, w_out):
    import ml_dtypes

    W = (np.asarray(ln_scale, np.float32)[:, None]
         * np.asarray(w_qkv, np.float32)).astype(ml_dtypes.bfloat16)
    cvec = (np.asarray(ln_bias, np.float32) @ np.asarray(w_qkv, np.float32)).astype(
        np.float32
    )
    wo = np.asarray(w_out, np.float32).astype(ml_dtypes.bfloat16)
    x = np.asarray(x, np.float32)
    return [
        {
            "x": np.ascontiguousarray(x[c * S:(c + 1) * S]),
            "wq": W,
            "cvec": cvec,
            "wo": wo,
        }
        for c in range(NC_CORES)
    ]


def kernel(x, ln_scale, ln_bias, w_qkv, w_out, b_out, **run_kwargs):
    in_maps = make_in_maps(x, ln_scale, ln_bias, w_qkv, w_out)
    nc = _get_graph()
    res = run_bass_kernel_spmd(
        nc, in_maps, core_ids=list(range(NC_CORES)), **run_kwargs
    )
    _GRAPH_CACHE["last_results"] = res
    out = np.concatenate(
        [np.asarray(r["out"]).T for r in res.results], axis=0
    )
    return (out + np.asarray(b_out, np.float32)).astype(np.float32)
